# revision 54
# baseline (speedup 1.0000x reference)
"""Two-layer GCN (BongardGNN) on 8 Trainium2 NeuronCores.

No usable data-dependent-addressing primitive exists in this toolchain,
so gathers run on the host (index-only reshuffles of device-produced
tables); all float arithmetic runs on device across three launches:

  L1 (8 cores): dis = 1/sqrt(1+deg) (deg as u8 table), q0 = dis*x -> fp16
  host: gather q0 rows into degree-region-packed, (j8 x f16)-interleaved
        feature-major slot rows; each node carries an extra self-slot.
  L2 (8 cores): agg = segment-reduce(slots)   [includes self term]
                s1  = dis*agg -> fp16
                h1  = relu(BDW1^T s1 + b1)    [block-diag weights on PE]
                q2  = dis*(BDW2^T h1) -> fp16
  host: gather q2 rows into (j64 x f2)-interleaved slot rows.
  L3 (8 cores): out = dis*segment-reduce(slots) + b2

Layout: nodes sorted by degree descending, grouped in 512s; regions of
equal max-degree get slot width d+1 (self slot) with near-zero padding.
Every core gets n_r/8 nodes of every region, so all 8 cores run one
identical program. Messages/no-accum tensors are fp16 (rel tol 2e-2
leaves ~30x margin); accumulation is f32 on device.
"""

import math
import os
import sys
import types

import numpy as np
import concourse.bacc as bacc
import concourse.tile as tile
from concourse import mybir
from concourse.bass_utils import run_bass_kernel_spmd

F32 = mybir.dt.float32
F16 = mybir.dt.float16
U8 = mybir.dt.uint8

TRACE = bool(os.environ.get("GNN_TRACE"))
LAST_EXEC_NS = []


def _enable_tracing():
    """Register the axon NTFF profile hook (absent from this image's antenv)
    and stub out the slow artifact upload. Test-time only (GNN_TRACE=1)."""
    if "antenv.axon_hooks" not in sys.modules:
        mod = types.ModuleType("antenv.axon_hooks")
        state = {}
        mod.set_axon_ntff_profile_hook = lambda h: state.update(h=h)
        mod.get_axon_ntff_profile_hook = lambda: state.get("h")
        sys.modules["antenv.axon_hooks"] = mod
        import antenv

        antenv.axon_hooks = mod
        sys.path.insert(0, "/root/.axon_site")
        from trn_agent_boot.trn_boot import _ntff_profile_via_ctypes

        mod.set_axon_ntff_profile_hook(
            _ntff_profile_via_ctypes("/opt/axon/libaxon_pjrt.so")
        )
    import concourse.bass_utils as bu

    bu.upload_artifacts = lambda tmpdir: "skipped"


def _run(nc, in_maps, core_ids):
    if TRACE:
        _enable_tracing()
        res = run_bass_kernel_spmd(nc, in_maps, core_ids=core_ids, trace=True)
        LAST_EXEC_NS.append(res.exec_time_ns)
        return res
    return run_bass_kernel_spmd(nc, in_maps, core_ids=core_ids)


N = 200000
D0, D1, D2 = 16, 32, 2
NCORES = 8
GRP = 512                    # node-group granularity: 8 cores x 64 j-subsets
SENT = N                     # sentinel node id -> guaranteed-zero table row
NPC1 = N // NCORES           # 25000 nodes/core in L1
K1 = 196                     # 25088 = 128*196 padded rows/core in L1
NPC1_PAD = 128 * K1
MAXC = 8192                  # L2 slot-stream tile columns (16KB/partition fp16)
CH = 512                     # matmul node-chunk (psum free-dim limit)


# --------------------------------------------------------------------------
# device programs
# --------------------------------------------------------------------------

def build_l1():
    """q0 = rsqrt(1+deg) * x, fp16 out. Node-major [128, K1*16] per core."""
    nc = bacc.Bacc("TRN2", target_bir_lowering=False, debug=False)
    xs = nc.dram_tensor("xs", [128, K1 * D0], F32, kind="ExternalInput")
    degp = nc.dram_tensor("degp", [128, K1], U8, kind="ExternalInput")
    q0 = nc.dram_tensor("q0", [128, K1 * D0], F16, kind="ExternalOutput")

    NCH = 4
    KC = K1 // NCH  # 49
    with tile.TileContext(nc) as tc:
        with (
            tc.tile_pool(name="cpool", bufs=1) as cpool,
            tc.tile_pool(name="pool", bufs=3) as pool,
        ):
            dsb = cpool.tile([128, K1], U8)
            nc.scalar.dma_start(out=dsb[:], in_=degp[:])
            dis = cpool.tile([128, K1], F32)
            nc.scalar.activation(
                dis[:], dsb[:],
                mybir.ActivationFunctionType.Abs_reciprocal_sqrt, bias=1.0,
            )
            # loads on the sync queue, stores on the scalar queue: each
            # HWDGE stream is FIFO in emission order, so a store's data
            # wait must not block the next chunk's load issue.
            for u in range(NCH):
                ks = slice(u * KC * D0, (u + 1) * KC * D0)
                xsb = pool.tile([128, KC * D0], F32, tag="xsb")
                nc.sync.dma_start(out=xsb[:], in_=xs[:, ks])
                q0sb = pool.tile([128, KC * D0], F16, tag="q0sb")
                nc.vector.tensor_tensor(
                    out=q0sb[:].rearrange("p (k f) -> p k f", f=D0),
                    in0=xsb[:].rearrange("p (k f) -> p k f", f=D0),
                    in1=dis[:, u * KC:(u + 1) * KC]
                    .rearrange("p (k o) -> p k o", o=1)
                    .to_broadcast([128, KC, D0]),
                    op=mybir.AluOpType.mult,
                )
                nc.scalar.dma_start(out=q0[:, ks], in_=q0sb[:])
    nc.compile()
    return nc


def _plan_stream(regions, maxc):
    """Pack per-row region spans (w, k) into DMA groups.

    Returns (groups, total_cols): groups = list of (group_cols, pieces),
    pieces = (w, k_take, col_off_in_group, node_off).
    """
    groups = []
    cur, cur_cols = [], 0
    node_off = 0
    for w, k in regions:
        kk = k
        while kk:
            take = min(kk, (maxc - cur_cols) // w)
            if take == 0:
                groups.append((cur_cols, cur))
                cur, cur_cols = [], 0
                continue
            cur.append((w, take, cur_cols, node_off))
            cur_cols += take * w
            node_off += take
            kk -= take
    if cur:
        groups.append((cur_cols, cur))
    total = sum(g[0] for g in groups)
    return groups, total


def build_l2(regions2, K8):
    """Slot-stream reduce + dis scale + both GCN matmuls, fp16 q2 out.

    Rows of every [128, *] tensor are (j8, f16): j-subset-major, feature
    minor. regions2: per-row (w, k) spans with k = n_r/64.
    """
    groups, S = _plan_stream(regions2, MAXC)
    nc = bacc.Bacc("TRN2", target_bir_lowering=False, debug=False)
    mg = nc.dram_tensor("mg", [128, S], F16, kind="ExternalInput")
    degA = nc.dram_tensor("degA", [128, K8], U8, kind="ExternalInput")
    degBlo = nc.dram_tensor("degBlo", [128, K8], U8, kind="ExternalInput")
    degBhi = nc.dram_tensor("degBhi", [128, K8], U8, kind="ExternalInput")
    bdw1lo = nc.dram_tensor("bdw1lo", [128, 128], F32, kind="ExternalInput")
    bdw1hi = nc.dram_tensor("bdw1hi", [128, 128], F32, kind="ExternalInput")
    bdw2 = nc.dram_tensor("bdw2", [128, 8], F32, kind="ExternalInput")
    b1p = nc.dram_tensor("b1p", [128, 1], F32, kind="ExternalInput")
    q2lo = nc.dram_tensor("q2lo", [8, K8], F16, kind="ExternalOutput")
    q2hi = nc.dram_tensor("q2hi", [8, K8], F16, kind="ExternalOutput")

    SEGC = 1024  # agg segment tiles let matmuls start before the last reduce
    nseg = max(1, K8 // SEGC)
    segk = [SEGC] * (nseg - 1) + [K8 - (nseg - 1) * SEGC]
    segoff = [SEGC * s for s in range(nseg)]

    def seg_of(n):
        return min(n // SEGC, nseg - 1)

    with tile.TileContext(nc) as tc:
        with (
            tc.tile_pool(name="stream", bufs=4) as spool,
            tc.tile_pool(name="cpool", bufs=1) as cpool,
            tc.tile_pool(name="work", bufs=2) as wpool,
            tc.tile_pool(name="psum", bufs=2, space="PSUM") as psum,
        ):
            # --- stream slot rows, segment-reduce into agg segments ---
            # (emitted first so the sync queue leads with the big mg loads;
            # small inputs ride the scalar-engine HWDGE queue below.)
            # fp16 tensor_tensor has a 2x uop while tensor_reduce is capped
            # at 1x, so fold slot pairs in-place (stride stays w, width
            # halves) before the final reduce.
            aggs = []
            for s in range(nseg):
                aggs.append(
                    cpool.tile(
                        [128, segk[s]], F32, name=f"agg{s}", tag=f"agg{s}"
                    )
                )

            def emit_reduce(mgt, w, k, coff, noff):
                wc = w
                halves = 1 + (1 if w % 8 == 0 else 0)
                if k * w < 2048:
                    halves = 0
                v = mgt[:, coff:coff + k * w].rearrange("p (k w) -> p k w", w=w)
                for _ in range(halves):
                    h = wc // 2
                    nc.vector.tensor_tensor(
                        out=v[:, :, :h], in0=v[:, :, :h], in1=v[:, :, h:wc],
                        op=mybir.AluOpType.add,
                    )
                    wc = h
                s = seg_of(noff)
                nc.vector.tensor_reduce(
                    out=aggs[s][:, noff - segoff[s]:noff - segoff[s] + k],
                    in_=v[:, :, :wc],
                    axis=mybir.AxisListType.X,
                    op=mybir.AluOpType.add,
                )

            # --- input DMA issues only (scalar queue, no compute between
            # them): every compute op on the scalar engine would delay the
            # issue of the early mg half-groups sharing this queue. The
            # ARS activations / weight casts are deferred until after the
            # stream loop — they are only needed by the matmul phase.
            def load_u8(nm, src_dram):
                dsb = cpool.tile([128, K8], U8, name=f"{nm}_u8", tag=f"{nm}_u8")
                nc.scalar.dma_start(out=dsb[:], in_=src_dram[:])
                return dsb

            dsbA = load_u8("disA", degA)
            dsbBlo = load_u8("disBlo", degBlo)
            dsbBhi = load_u8("disBhi", degBhi)

            def load_w(nm, src, shape):
                t32 = cpool.tile(shape, F32, name=f"{nm}_32", tag=f"{nm}_32")
                nc.scalar.dma_start(out=t32[:], in_=src[:])
                return t32

            w1lo32 = load_w("w1lo", bdw1lo, [128, 128])
            w1hi32 = load_w("w1hi", bdw1hi, [128, 128])
            w2_32 = load_w("w2", bdw2, [128, 8])
            b1sb = cpool.tile([128, 1], F32)
            nc.scalar.dma_start(out=b1sb[:], in_=b1p[:])

            goff = 0
            for gi, (cols, pieces) in enumerate(groups):
                mgt = spool.tile([128, MAXC], F16, tag="mgt")
                if gi < 5:
                    # early groups: column-split across both HWDGE queues
                    # to halve arrival latency (ACT is still idle here, so
                    # its queue is free; late groups must not ride it).
                    half = cols // 2
                    nc.sync.dma_start(
                        out=mgt[:, :half], in_=mg[:, goff:goff + half]
                    )
                    nc.scalar.dma_start(
                        out=mgt[:, half:cols],
                        in_=mg[:, goff + half:goff + cols],
                    )
                else:
                    nc.sync.dma_start(
                        out=mgt[:, :cols], in_=mg[:, goff:goff + cols]
                    )
                for w, k, coff, noff in pieces:
                    done = 0
                    while done < k:  # split pieces at segment boundaries
                        g0 = noff + done
                        s = seg_of(g0)
                        seg_end = segoff[s] + segk[s]
                        take = min(k - done, seg_end - g0)
                        emit_reduce(mgt, w, take, coff + done * w, g0)
                        done += take
                goff += cols

            # --- deferred: dis tables (1/sqrt(deg+1)) and weight casts ---
            def make_dis(nm, dsb, out_dtype):
                dh = cpool.tile([128, K8], out_dtype, name=f"{nm}_h", tag=f"{nm}_h")
                nc.scalar.activation(
                    dh[:], dsb[:],
                    mybir.ActivationFunctionType.Abs_reciprocal_sqrt, bias=1.0,
                )
                return dh

            disA = make_dis("disA", dsbA, F32)
            disBlo = make_dis("disBlo", dsbBlo, F16)
            disBhi = make_dis("disBhi", dsbBhi, F16)

            def cast_w(nm, t32, shape):
                t16 = cpool.tile(shape, F16, name=f"{nm}_16", tag=f"{nm}_16")
                nc.vector.tensor_copy(out=t16[:], in_=t32[:])
                return t16

            w1lo = cast_w("w1lo", w1lo32, [128, 128])
            w1hi = cast_w("w1hi", w1hi32, [128, 128])
            w2 = cast_w("w2", w2_32, [128, 8])

            # --- per segment: s1 = disA*agg, then 16->32 relu dis 32->2 ---
            q2sb = {}
            for h in (0, 1):
                q2sb[h] = cpool.tile(
                    [8, K8], F16, tag=f"q2sb{h}", name=f"q2sb{h}"
                )
            for s in range(nseg):
                soff = segoff[s]
                s1f = wpool.tile(
                    [128, max(segk)], F16, tag="s1f", name=f"s1f{s}"
                )
                nc.vector.tensor_tensor(
                    out=s1f[:, :segk[s]], in0=aggs[s][:],
                    in1=disA[:, soff:soff + segk[s]], op=mybir.AluOpType.mult,
                )
                for h, w1h, disB in ((0, w1lo, disBlo), (1, w1hi, disBhi)):
                    for c0 in range(0, segk[s], CH):
                        m = min(CH, segk[s] - c0)
                        g0 = soff + c0
                        h1p = psum.tile([128, CH], F32, tag="h1p", bufs=3)
                        nc.tensor.matmul(
                            out=h1p[:, :m], lhsT=w1h[:],
                            rhs=s1f[:, c0:c0 + m], start=True, stop=True,
                        )
                        h1d = wpool.tile([128, CH], F16, tag="h1d", bufs=3)
                        nc.scalar.activation(
                            h1d[:, :m], h1p[:, :m],
                            mybir.ActivationFunctionType.Relu, bias=b1sb[:],
                        )
                        nc.vector.tensor_tensor(
                            out=h1d[:, :m], in0=h1d[:, :m],
                            in1=disB[:, g0:g0 + m], op=mybir.AluOpType.mult,
                        )
                        q2p = psum.tile([8, CH], F32, tag="q2p")
                        nc.tensor.matmul(
                            out=q2p[:, :m], lhsT=w2[:], rhs=h1d[:, :m],
                            start=True, stop=True,
                        )
                        nc.scalar.copy(
                            out=q2sb[h][:, g0:g0 + m], in_=q2p[:, :m]
                        )
            nc.scalar.dma_start(out=q2lo[:], in_=q2sb[0][:])
            nc.scalar.dma_start(out=q2hi[:], in_=q2sb[1][:])
    nc.compile()
    return nc, S


def build_l3(regions3, K64):
    """Second-layer slot reduce + dis scale + bias. Rows are (j64, f2)."""
    groups, S3 = _plan_stream(regions3, 2048)
    nc = bacc.Bacc("TRN2", target_bir_lowering=False, debug=False)
    mg2 = nc.dram_tensor("mg2", [128, S3], F16, kind="ExternalInput")
    degC = nc.dram_tensor("degC", [128, K64], U8, kind="ExternalInput")
    b2p = nc.dram_tensor("b2p", [128, 1], F32, kind="ExternalInput")
    outT = nc.dram_tensor("outT", [128, K64], F32, kind="ExternalOutput")

    with tile.TileContext(nc) as tc:
        with (
            tc.tile_pool(name="stream", bufs=3) as spool,
            tc.tile_pool(name="cpool", bufs=1) as cpool,
        ):
            dsb = cpool.tile([128, K64], U8)
            nc.scalar.dma_start(out=dsb[:], in_=degC[:])
            disC = cpool.tile([128, K64], F32)
            nc.scalar.activation(
                disC[:], dsb[:],
                mybir.ActivationFunctionType.Abs_reciprocal_sqrt, bias=1.0,
            )
            b2sb = cpool.tile([128, 1], F32)
            nc.scalar.dma_start(out=b2sb[:], in_=b2p[:])

            agg = cpool.tile([128, K64], F32)
            goff = 0
            for gi, (cols, pieces) in enumerate(groups):
                mgt = spool.tile([128, 2048], F16, tag="mgt")
                nc.sync.dma_start(
                    out=mgt[:, :cols], in_=mg2[:, goff:goff + cols]
                )
                for w, k, coff, noff in pieces:
                    wc = w
                    halves = 1 + (1 if w % 8 == 0 else 0)
                    if k * w < 2048:
                        halves = 0
                    v = mgt[:, coff:coff + k * w].rearrange(
                        "p (k w) -> p k w", w=w
                    )
                    for _ in range(halves):
                        h = wc // 2
                        nc.vector.tensor_tensor(
                            out=v[:, :, :h], in0=v[:, :, :h],
                            in1=v[:, :, h:wc], op=mybir.AluOpType.add,
                        )
                        wc = h
                    nc.vector.tensor_reduce(
                        out=agg[:, noff:noff + k],
                        in_=v[:, :, :wc],
                        axis=mybir.AxisListType.X,
                        op=mybir.AluOpType.add,
                    )
                goff += cols

            nc.vector.tensor_tensor(
                out=agg[:], in0=agg[:], in1=disC[:], op=mybir.AluOpType.mult
            )
            osb = cpool.tile([128, K64], F32)
            nc.scalar.activation(
                osb[:], agg[:], mybir.ActivationFunctionType.Identity,
                bias=b2sb[:],
            )
            nc.sync.dma_start(out=outT[:], in_=osb[:])
    nc.compile()
    return nc, S3


# --------------------------------------------------------------------------
# host-side index machinery (static given edge_index)
# --------------------------------------------------------------------------

def build_layout(deg):
    """Degree-descending node permutation in 512-groups; equal-max-degree
    groups merge into regions of slot width d+1 (self slot included)."""
    order = np.argsort(-deg, kind="stable")
    ntot = ((N + GRP - 1) // GRP) * GRP
    perm = np.full(ntot, SENT, np.int64)
    perm[:N] = order
    dsorted = np.zeros(ntot, np.int64)
    dsorted[:N] = deg[order]
    lead = dsorted[::GRP]
    regions = []  # (w, n_nodes); w = slots/node (neighbors + self), mult of 4
    for g in range(len(lead)):
        w = ((int(lead[g]) + 1 + 3) // 4) * 4
        if regions and regions[-1][0] == w:
            regions[-1][1] += GRP
        else:
            regions.append([w, GRP])
    return perm, [(w, n) for w, n in regions], ntot


def build_slot_tables(src, dst, deg, perm, regions):
    """Per-region slot-source tables [n_r, w_r] int32 (self slot last)."""
    order_e = np.argsort(dst, kind="stable")
    s_src = src[order_e].astype(np.int32)
    E = len(s_src)
    starts = np.zeros(N + 1, np.int64)
    np.cumsum(deg, out=starts[1:])
    tables = []
    off = 0
    for w, n_r in regions:
        nodes = perm[off:off + n_r]
        off += n_r
        real = nodes < N
        base = np.where(real, starts[np.minimum(nodes, N - 1)], 0)
        dgs = np.where(real, deg[np.minimum(nodes, N - 1)], 0)
        cols = np.arange(w - 1, dtype=np.int64)
        gat = np.minimum(base[:, None] + cols[None, :], E - 1)
        M = s_src[gat]
        M[cols[None, :] >= dgs[:, None]] = SENT
        slot = np.concatenate(
            [M, np.where(real, nodes, SENT)[:, None].astype(np.int32)], axis=1
        )
        tables.append(slot)
    return tables


def gather_rows(tab, tables, regions, jn, nf):
    """Build per-core [jn*nf, S] slot rows from table lookups.

    tab: [N+1, nf] source table (row SENT is zero).
    Returns [8, jn*nf, S] contiguous array of tab.dtype.
    """
    blocks = []
    for (w, n_r), slot in zip(regions, tables):
        k = n_r // (NCORES * jn)
        ids = slot.reshape(NCORES, jn, k, w)
        G = tab[ids]                                  # [8, jn, k, w, nf]
        blocks.append(
            G.transpose(0, 1, 4, 2, 3).reshape(NCORES, jn * nf, k * w)
        )
    return np.ascontiguousarray(np.concatenate(blocks, axis=2))


def node_ids(perm, regions, jn):
    """Per-region node-id arrays [8, jn, k] in layout order."""
    out = []
    off = 0
    for w, n_r in regions:
        k = n_r // (NCORES * jn)
        out.append(perm[off:off + n_r].reshape(NCORES, jn, k))
        off += n_r
    return out


def deg_rows(deg_ext, perm, regions, jn, rep):
    """Per-core [jn*rep, K] node-degree rows (repeated rep x per node)."""
    blocks = []
    for ids in node_ids(perm, regions, jn):
        dd = deg_ext[ids]                             # [8, jn, k]
        k = dd.shape[2]
        blocks.append(
            np.repeat(dd[:, :, None, :], rep, axis=2).reshape(
                NCORES, jn * rep, k
            )
        )
    return np.ascontiguousarray(np.concatenate(blocks, axis=2))


# --------------------------------------------------------------------------
# entry point
# --------------------------------------------------------------------------

def kernel(x, edge_index, W1, b1, W2, b2):
    LAST_EXEC_NS.clear()
    x = np.asarray(x, np.float32)
    W1 = np.asarray(W1, np.float32)
    b1 = np.asarray(b1, np.float32)
    W2 = np.asarray(W2, np.float32)
    b2 = np.asarray(b2, np.float32)
    src = np.asarray(edge_index[0], np.int64)
    dst = np.asarray(edge_index[1], np.int64)

    deg = np.bincount(dst, minlength=N).astype(np.int64)
    assert deg.max() <= 255, f"max degree {deg.max()} exceeds uint8"
    deg_ext = np.zeros(N + 1, np.uint8)
    deg_ext[:N] = deg

    perm, regions, ntot = build_layout(deg)
    K8 = ntot // 64
    K64 = ntot // GRP
    tables = build_slot_tables(src, dst, deg, perm, regions)
    regions2 = [(w, n // 64) for w, n in regions]
    regions3 = [(w, n // GRP) for w, n in regions]

    # ---- L1: q0 = dis * x (node-major, original ids, 8-way split) ----
    l1 = build_l1()
    in1 = []
    for c in range(NCORES):
        xp = np.zeros((NPC1_PAD, D0), np.float32)
        xp[:NPC1] = x[c * NPC1:(c + 1) * NPC1]
        dp = np.zeros(NPC1_PAD, np.uint8)
        dp[:NPC1] = deg_ext[c * NPC1:(c + 1) * NPC1]
        in1.append(
            {
                "xs": np.ascontiguousarray(xp.reshape(128, K1 * D0)),
                "degp": np.ascontiguousarray(dp.reshape(128, K1)),
            }
        )
    r1 = _run(l1, in1, core_ids=list(range(NCORES))).results
    q0tab = np.zeros((N + 1, D0), np.float16)
    for c in range(NCORES):
        q0tab[c * NPC1:(c + 1) * NPC1] = (
            np.asarray(r1[c]["q0"]).reshape(NPC1_PAD, D0)[:NPC1]
        )

    # ---- L2 ----
    l2, S = build_l2(regions2, K8)
    mg1 = gather_rows(q0tab, tables, regions, 8, D0)
    assert mg1.shape[2] == S
    degA = deg_rows(deg_ext, perm, regions, 8, D0)
    degB = deg_rows(deg_ext, perm, regions, 8, D1)    # [8, 256, K8]
    bdw1lo = np.zeros((128, 128), np.float32)
    bdw1hi = np.zeros((128, 128), np.float32)
    for j in range(4):
        bdw1lo[j * 16:(j + 1) * 16, j * 32:(j + 1) * 32] = W1
        bdw1hi[(j + 4) * 16:(j + 5) * 16, j * 32:(j + 1) * 32] = W1
    bdw2 = np.zeros((128, 8), np.float32)
    for j in range(4):
        bdw2[j * 32:(j + 1) * 32, j * 2:(j + 1) * 2] = W2
    b1p = np.ascontiguousarray(np.tile(b1, 4).reshape(128, 1))
    in2 = []
    for c in range(NCORES):
        in2.append(
            {
                "mg": mg1[c],
                "degA": degA[c],
                "degBlo": np.ascontiguousarray(degB[c, :128]),
                "degBhi": np.ascontiguousarray(degB[c, 128:]),
                "bdw1lo": bdw1lo,
                "bdw1hi": bdw1hi,
                "bdw2": bdw2,
                "b1p": b1p,
            }
        )
    r2 = _run(l2, in2, core_ids=list(range(NCORES))).results
    del mg1, degA, degB

    # q2 descramble -> [N+1, 2] fp16 table
    q2tab = np.zeros((N + 1, D2), np.float16)
    ids2 = node_ids(perm, regions, 8)
    noff = 0
    q2v = [
        (np.asarray(r2[c]["q2lo"]), np.asarray(r2[c]["q2hi"]))
        for c in range(NCORES)
    ]
    for ids in ids2:
        k = ids.shape[2]
        for c in range(NCORES):
            lo = q2v[c][0][:, noff:noff + k].reshape(4, 2, k)
            hi = q2v[c][1][:, noff:noff + k].reshape(4, 2, k)
            vals = np.concatenate([lo, hi], axis=0).transpose(0, 2, 1)
            q2tab[ids[c]] = vals
        noff += k
    q2tab[N:] = 0.0

    # ---- L3 ----
    l3, S3 = build_l3(regions3, K64)
    mg2 = gather_rows(q2tab, tables, regions, 64, D2)
    assert mg2.shape[2] == S3
    degC = deg_rows(deg_ext, perm, regions, 64, D2)
    b2p = np.ascontiguousarray(np.tile(b2, 64).reshape(128, 1))
    in3 = [
        {"mg2": mg2[c], "degC": degC[c], "b2p": b2p} for c in range(NCORES)
    ]
    r3 = _run(l3, in3, core_ids=list(range(NCORES))).results

    out_full = np.zeros((N + 1, D2), np.float32)
    ids3 = node_ids(perm, regions, 64)
    noff = 0
    for ids in ids3:
        k = ids.shape[2]
        for c in range(NCORES):
            v = np.asarray(r3[c]["outT"])[:, noff:noff + k]
            out_full[ids[c]] = v.reshape(64, 2, k).transpose(0, 2, 1)
        noff += k
    return np.ascontiguousarray(out_full[:N])
